# revision 1
# baseline (speedup 1.0000x reference)
"""DeepseekV2 MLA decode attention on 8 Trainium2 NeuronCores.

Strategy (single SPMD launch, identical program on all cores; all per-core
variation comes from in_maps contents and collective semantics):

  - Attention is batch-sharded: core k owns sequences 4k..4k+4, whose KV
    cache slices are fed to it via in_maps in TWO host-prepared layouts:
    natural [s, c] (context matmul, contracts s) and transposed [c, s]
    (score matmul, contracts c). The PE contracts along partitions, so the
    two matmuls need different partition assignments of the same data;
    host-side dual layout avoids all on-chip cache transposition.
  - Scores are computed transposed (PSUM [128 s, 16 h]) so the exp (ACT)
    writes e_T directly in the layout the context matmul consumes as its
    stationary operand.
  - w_qkv_a is K-sharded (hidden dim / 8); the row-major partial qkv
    activations are ReduceScattered, which both sums the partials and
    hands each core exactly its 4 sequences (rank-dependent slicing via
    collective semantics).
  - w_o is column-sharded; ctx_v rows are AllGathered and each core
    produces a 640-column slice of the output, concatenated on host.
  - q_a_norm_w is folded into w_q_b on the host (rmsnorm scale is diag).
  - The current-token cache update (rmsnorm latent / roped k_pe written
    at slot S-1) is applied on the host while building the cache layouts.
"""

import sys

sys.path.insert(0, "/opt/trn_rl_repo")

import numpy as np

import concourse.bacc as bacc
import concourse.mybir as mybir
import concourse.tile as tile
from concourse import bass_utils
from concourse.masks import make_identity

F32 = mybir.dt.float32
ADD = mybir.AluOpType.add
MULT = mybir.AluOpType.mult
BYPASS = mybir.AluOpType.bypass
EXP = mybir.ActivationFunctionType.Exp
SQRT = mybir.ActivationFunctionType.Sqrt
AXIS_X = mybir.AxisListType.X

B, HID, H = 32, 5120, 16
DN, DR, DV = 128, 64, 128
QL, KL = 1536, 512
BASE = 10000.0
EPS = 1e-6
SCALE = float((DN + DR) ** -0.5)

N_CORES = 8
BP = B // N_CORES      # sequences per core
NKT = QL // 128        # 12
TP = True              # collective-based weight sharding

_CACHE = {}


# ----------------------------- host math ---------------------------------


def _rmsnorm_np(x, w):
    ms = np.mean(x * x, axis=-1, keepdims=True, dtype=np.float32)
    return (x * (1.0 / np.sqrt(ms + EPS)) * w).astype(np.float32)


def _rope_np(x, pos):
    d = x.shape[-1]
    inv = (1.0 / (BASE ** (np.arange(0, d, 2, dtype=np.float32) / d))).astype(
        np.float32
    )
    fr = pos.astype(np.float32)[:, None] * inv
    cos, sin = np.cos(fr).astype(np.float32), np.sin(fr).astype(np.float32)
    out = np.empty_like(x)
    out[..., 0::2] = x[..., 0::2] * cos - x[..., 1::2] * sin
    out[..., 1::2] = x[..., 1::2] * cos + x[..., 0::2] * sin
    return out.astype(np.float32)


def _rope_RT(pos):
    """Per-batch transposed rotation matrices (lhsT for rope-as-matmul)."""
    inv = (1.0 / (BASE ** (np.arange(0, DR, 2, dtype=np.float32) / DR))).astype(
        np.float32
    )
    fr = pos.astype(np.float32)[:, None] * inv
    cos, sin = np.cos(fr).astype(np.float32), np.sin(fr).astype(np.float32)
    R = np.zeros((B, DR, DR), np.float32)
    j = np.arange(DR // 2)
    bi = np.arange(B)[:, None]
    R[bi, 2 * j, 2 * j] = cos
    R[bi, 2 * j, 2 * j + 1] = -sin
    R[bi, 2 * j + 1, 2 * j] = sin
    R[bi, 2 * j + 1, 2 * j + 1] = cos
    return np.ascontiguousarray(R.transpose(0, 2, 1))


# ----------------------------- device program ----------------------------


def _build(S, n_cores, tp, fake_coll=False, TRF=2):
    nc = bacc.Bacc("TRN2", target_bir_lowering=False, debug=False,
                   enable_asserts=False, num_devices=n_cores)
    ST = S // 512
    rg = [list(range(n_cores))]
    NB = B if tp else BP            # batch width of the qkv_a projection
    KTH = (HID // n_cores if tp else HID) // 128   # hidden k-tiles (5 / 40)
    HO = HID // n_cores if tp else HID             # output columns per core

    hT = nc.dram_tensor("hT", [128 * KTH, NB], F32, kind="ExternalInput")
    w_qa = nc.dram_tensor("w_qa", [128 * KTH, QL], F32, kind="ExternalInput")
    w_qb = nc.dram_tensor("w_qb", [QL, H * (DN + DR)], F32,
                          kind="ExternalInput")
    w_kc = nc.dram_tensor("w_kc", [H, DN, KL], F32, kind="ExternalInput")
    w_vc = nc.dram_tensor("w_vc", [H, KL, DV], F32, kind="ExternalInput")
    w_o = nc.dram_tensor("w_o", [H * DV, HO], F32, kind="ExternalInput")
    cache_nat = nc.dram_tensor("cache_nat", [BP, S, KL], F32,
                               kind="ExternalInput")
    cacheT_l = nc.dram_tensor("cacheT_l", [BP, KL, S], F32,
                              kind="ExternalInput")
    cacheT_r = nc.dram_tensor("cacheT_r", [BP, DR, S], F32,
                              kind="ExternalInput")
    ropeRT = nc.dram_tensor("ropeRT", [BP, DR, DR], F32, kind="ExternalInput")
    out = nc.dram_tensor("out", [NB if tp else BP, HO], F32,
                         kind="ExternalOutput")

    with tile.TileContext(nc) as tc:
        with (
            tc.tile_pool(name="const", bufs=1) as cp,
            tc.tile_pool(name="qsb", bufs=1) as qsb,
            tc.tile_pool(name="dram", bufs=1, space="DRAM") as dramp,
            tc.tile_pool(name="wstream", bufs=2) as wsp,
            tc.tile_pool(name="wo", bufs=1) as wop,
            tc.tile_pool(name="ctl", bufs=3) as ctlp,
            tc.tile_pool(name="ctr", bufs=1) as ctrp,
            tc.tile_pool(name="nat", bufs=4) as natp,
            tc.tile_pool(name="et", bufs=4) as etp,
            tc.tile_pool(name="small", bufs=1) as smp,
            tc.tile_pool(name="small2", bufs=2) as smp2,
        ):
            ones_col = cp.tile([128, 1], F32)
            nc.any.memset(ones_col, 1.0)
            eps_t = cp.tile([128, 1], F32)
            nc.any.memset(eps_t, EPS)
            ident = cp.tile([128, 128], F32)
            make_identity(nc, ident[:, :])
            rt_sb = cp.tile([DR, BP, DR], F32)
            nc.sync.dma_start(rt_sb[:, :, :],
                              ropeRT[:, :, :].rearrange("b k m -> k b m"))
            hT_sb = cp.tile([128, KTH, NB], F32)
            nc.sync.dma_start(hT_sb[:, :, :],
                              hT[:, :].rearrange("(t p) b -> p t b", p=128))

            # ================= q path =================
            with tc.tile_pool(name="psq", bufs=6, space="PSUM") as psq:

                def qps(name):
                    return psq.tile([128, 512], F32, tag="q", name=name)

                # ---- qkv_a projection: q_a rows [NB, 1536] ----
                qkv_rows = qsb.tile([NB, QL], F32)
                pss = [qps(f"qkv{j}") for j in range(3)]
                for kt in range(KTH):
                    wt = wsp.tile([128, 1536], F32, tag="wqa")
                    nc.sync.dma_start(wt[:, :],
                                      w_qa[kt * 128:(kt + 1) * 128, :])
                    for j in range(3):
                        nc.tensor.matmul(
                            pss[j][:NB, :], hT_sb[:, kt, :],
                            wt[:, j * 512:(j + 1) * 512],
                            start=(kt == 0), stop=(kt == KTH - 1))
                for j in range(3):
                    nc.any.tensor_copy(
                        qkv_rows[:, j * 512:(j + 1) * 512], pss[j][:NB, :])

                # ---- ReduceScatter partials -> my 4 sequences' q_a ----
                if tp:
                    rs_in = dramp.tile([B, QL], F32)
                    rs_out = dramp.tile([BP, QL], F32)
                    nc.sync.dma_start(rs_in[:, :], qkv_rows[:, :])
                    if fake_coll:
                        nc.sync.dma_start(rs_out[:, :], rs_in[0:BP, :])
                    else:
                        nc.gpsimd.collective_compute(
                            "ReduceScatter", ADD, replica_groups=rg,
                            ins=[rs_in.opt()], outs=[rs_out.opt()])
                    qa_mine = qsb.tile([BP, QL], F32)
                    nc.sync.dma_start(qa_mine[:, :], rs_out[:, :])
                else:
                    qa_mine = qkv_rows

                # ---- rmsnorm (rows) + transpose to [128, 12, 4] ----
                sq = smp.tile([BP, QL], F32, tag="sq")
                nc.vector.tensor_tensor(sq[:, :], qa_mine[:, :],
                                        qa_mine[:, :], MULT)
                ssum = smp.tile([BP, 1], F32, tag="ssum")
                nc.vector.reduce_sum(ssum[:, :], sq[:, :], AXIS_X)
                rms = smp.tile([BP, 1], F32, tag="rms")
                nc.scalar.activation(rms[:, :], ssum[:, :], SQRT,
                                     bias=eps_t[:BP, :1], scale=1.0 / QL)
                rinv = smp.tile([BP, 1], F32, tag="rinv")
                nc.vector.reciprocal(rinv[:, :], rms[:, :])
                qan = smp.tile([BP, QL], F32, tag="qan")
                nc.vector.tensor_scalar_mul(qan[:, :], qa_mine[:, :],
                                            rinv[:, :1])

                ps_t = qps("qanT")
                for t in range(NKT):
                    nc.tensor.transpose(ps_t[:, t * BP:(t + 1) * BP],
                                        qan[:BP, t * 128:(t + 1) * 128],
                                        ident[:BP, :BP])
                qanT = qsb.tile([128, NKT, BP], F32)
                nc.any.tensor_copy(qanT[:, :, :], ps_t[:, :NKT * BP])

                # ---- q_b (norm weight folded in) per head -> nope/pe ----
                ps_n = qps("qbn")
                ps_p = qps("qbp")
                for h in range(H):
                    wt = wsp.tile([128, NKT, DN + DR], F32, tag="wqb")
                    nc.sync.dma_start(
                        wt[:, :, :],
                        w_qb[:, h * (DN + DR):(h + 1) * (DN + DR)]
                        .rearrange("(t p) m -> p t m", p=128))
                    for t in range(NKT):
                        nc.tensor.matmul(ps_n[:, h * BP:(h + 1) * BP],
                                         wt[:, t, :DN], qanT[:, t, :],
                                         start=(t == 0), stop=(t == NKT - 1))
                    for t in range(NKT):
                        nc.tensor.matmul(ps_p[:64, h * BP:(h + 1) * BP],
                                         wt[:, t, DN:], qanT[:, t, :],
                                         start=(t == 0), stop=(t == NKT - 1))
                qnopeT = qsb.tile([128, H, BP], F32)
                nc.any.tensor_copy(qnopeT[:, :, :],
                                   ps_n[:, :H * BP]
                                   .rearrange("p (h b) -> p h b", h=H))
                qpe_raw = qsb.tile([64, H, BP], F32)
                nc.any.tensor_copy(qpe_raw[:, :, :],
                                   ps_p[:64, :H * BP]
                                   .rearrange("p (h b) -> p h b", h=H))

                # ---- rope(q_pe) as matmul with per-batch rotation ----
                ps_r = qps("rope")
                for h in range(H):
                    for b in range(BP):
                        nc.tensor.matmul(
                            ps_r[:64, h * BP + b:h * BP + b + 1],
                            rt_sb[:, b, :], qpe_raw[:, h, b:b + 1],
                            start=True, stop=True)
                qpeT = qsb.tile([64, H, BP], F32)
                nc.any.tensor_copy(qpeT[:, :, :],
                                   ps_r[:64, :H * BP]
                                   .rearrange("p (h b) -> p h b", h=H))

                # ---- absorb q_nope through w_kc: qabsT [128, 4, H, BP] ----
                ps_a = [qps(f"abs{c}") for c in range(4)]
                for h in range(H):
                    kt_ = wsp.tile([128, KL], F32, tag="wkc")
                    nc.sync.dma_start(kt_[:, :], w_kc[h, :, :])
                    for c in range(4):
                        nc.tensor.matmul(ps_a[c][:, h * BP:(h + 1) * BP],
                                         kt_[:, c * 128:(c + 1) * 128],
                                         qnopeT[:, h, :],
                                         start=True, stop=True)
                qabsT = qsb.tile([128, 4, H, BP], F32)
                for c in range(4):
                    nc.any.tensor_copy(qabsT[:, c, :, :],
                                       ps_a[c][:, :H * BP]
                                       .rearrange("p (h b) -> p h b", h=H))

            # ================= attention =================
            wvc_res = qsb.tile([128, H, 4, DV], F32)
            nc.sync.dma_start(
                wvc_res[:, :, :, :],
                w_vc[:, :, :].rearrange("h (c p) v -> p h c v", p=128))
            ctxT = qsb.tile([128, 4, H, BP], F32)
            with (
                tc.tile_pool(name="pssc", bufs=2, space="PSUM") as pssc,
                tc.tile_pool(name="psctx", bufs=2, space="PSUM") as psctx,
                tc.tile_pool(name="pssum", bufs=1, space="PSUM") as pssum,
                tc.tile_pool(name="psctt", bufs=1, space="PSUM") as psctt,
                tc.tile_pool(name="pstr", bufs=2, space="PSUM") as pstr,
            ):
                sums = pssum.tile([16, BP], F32, tag="sums")

                def attn_seq(lb, ctx_ps):
                    seq_ctr = [None]
                    for st in range(ST):
                        s0 = st * 512
                        ctl = ctlp.tile([128, 4, 512], F32, tag="ctl")
                        nc.sync.dma_start(
                            ctl[:, :, :],
                            cacheT_l[lb, :, s0:s0 + 512]
                            .rearrange("(t p) s -> p t s", p=128))
                        if st == 0:
                            ctr_seq = ctrp.tile([64, S], F32, tag="ctr")
                            nc.sync.dma_start(ctr_seq[:, :],
                                              cacheT_r[lb, :, :])
                            seq_ctr[0] = ctr_seq
                        ctr = seq_ctr[0][:, s0:s0 + 512]
                        sc = pssc.tile([128, 4 * H], F32, tag="sc")
                        for i in range(4):
                            for c in range(4):
                                nc.tensor.matmul(
                                    sc[:, i * H:(i + 1) * H],
                                    ctl[:, c, i * 128:(i + 1) * 128],
                                    qabsT[:, c, :, lb],
                                    start=(c == 0), stop=False)
                            nc.tensor.matmul(
                                sc[:, i * H:(i + 1) * H],
                                ctr[:, i * 128:(i + 1) * 128],
                                qpeT[:, :, lb], start=False, stop=True)
                        eT = etp.tile([128, 4 * H], F32, tag="eT")
                        nc.scalar.activation(eT[:, :], sc[:, :], EXP,
                                             scale=SCALE)
                        for i in range(4):
                            # natural-layout chunk: PE-transpose the resident
                            # [c, s] tile for TRF of 4 chunks, stream the
                            # rest from the host natural layout -- balances
                            # the HBM-read saving against PE transpose cost
                            natc = natp.tile([128, KL], F32, tag="nat")
                            if i < TRF:
                                ps_tr = pstr.tile([128, KL], F32, tag="tr")
                                for c in range(4):
                                    nc.tensor.transpose(
                                        ps_tr[:, c * 128:(c + 1) * 128],
                                        ctl[:, c, i * 128:(i + 1) * 128],
                                        ident[:, :])
                                nc.scalar.copy(natc[:, :], ps_tr[:, :])
                            else:
                                nc.sync.dma_start(
                                    natc[:, :],
                                    cache_nat[lb,
                                              s0 + i * 128:s0 + (i + 1) * 128,
                                              :])
                            nc.tensor.matmul(
                                ctx_ps[:16, :], eT[:, i * H:(i + 1) * H],
                                natc[:, :],
                                start=(st == 0 and i == 0),
                                stop=(st == ST - 1 and i == 3))
                            nc.tensor.matmul(
                                sums[:16, lb:lb + 1],
                                eT[:, i * H:(i + 1) * H], ones_col[:, :1],
                                start=(st == 0 and i == 0),
                                stop=(st == ST - 1 and i == 3))

                for lb in range(BP):
                    ctx_ps = psctx.tile([16, KL], F32, tag="ctx",
                                        name=f"ctx{lb}")
                    attn_seq(lb, ctx_ps)
                    rec = smp2.tile([16, 1], F32, tag="rec")
                    nc.vector.reciprocal(rec[:, :], sums[:16, lb:lb + 1])
                    ctxn = smp2.tile([16, KL], F32, tag="ctxn")
                    nc.vector.tensor_scalar_mul(ctxn[:, :], ctx_ps[:16, :],
                                                rec[:, :1])
                    ps_ct = psctt.tile([128, 4 * H], F32, tag="ctxT")
                    for c in range(4):
                        nc.tensor.transpose(ps_ct[:, c * H:(c + 1) * H],
                                            ctxn[:16, c * 128:(c + 1) * 128],
                                            ident[:16, :16])
                    nc.any.tensor_copy(
                        ctxT[:, :, :, lb],
                        ps_ct[:, :].rearrange("p (c h) -> p c h", c=4))

                # ---- un-absorb values: ovT [128 v, H, BP] ----
                ps_v = pssum.tile([128, H * BP], F32, tag="sums",
                                  name="ps_v")
                for h in range(H):
                    for c in range(4):
                        nc.tensor.matmul(ps_v[:, h * BP:(h + 1) * BP],
                                         wvc_res[:, h, c, :], ctxT[:, c, h, :],
                                         start=(c == 0), stop=(c == 3))
                ovT = qsb.tile([128, H, BP], F32)
                nc.any.tensor_copy(ovT[:, :, :],
                                   ps_v[:, :]
                                   .rearrange("p (h b) -> p h b", h=H))

            # ================= output projection =================
            with (
                tc.tile_pool(name="psor", bufs=1, space="PSUM") as psor,
                tc.tile_pool(name="psot", bufs=1, space="PSUM") as psot,
                tc.tile_pool(name="psoo", bufs=2, space="PSUM") as psoo,
            ):
                if tp:
                    # ovT -> rows [4, 2048] -> AllGather -> [32, 2048] -> T
                    ps_rows = psor.tile([BP, H * DV], F32, tag="ovr")
                    for h in range(H):
                        nc.tensor.transpose(
                            ps_rows[:BP, h * DV:(h + 1) * DV],
                            ovT[:, h, :], ident[:, :])
                    ov_rows = smp.tile([BP, H * DV], F32, tag="ovrows")
                    nc.any.tensor_copy(ov_rows[:, :], ps_rows[:BP, :])
                    agv_in = dramp.tile([BP, H * DV], F32)
                    agv_out = dramp.tile([B, H * DV], F32)
                    nc.sync.dma_start(agv_in[:, :], ov_rows[:, :])
                    if fake_coll:
                        nc.sync.dma_start(agv_out[0:BP, :], agv_in[:, :])
                    else:
                        nc.gpsimd.collective_compute(
                            "AllGather", BYPASS, replica_groups=rg,
                            ins=[agv_in.opt()], outs=[agv_out.opt()])
                    ov32 = smp.tile([B, H * DV], F32, tag="ov32")
                    nc.sync.dma_start(ov32[:, :], agv_out[:, :])
                    ps_tt = psot.tile([128, 16 * B], F32, tag="ovtt")
                    for kt in range(16):
                        nc.tensor.transpose(
                            ps_tt[:, kt * B:(kt + 1) * B],
                            ov32[:B, kt * 128:(kt + 1) * 128], ident[:B, :B])
                    ovT_f = qsb.tile([128, 16, B], F32)
                    nc.any.tensor_copy(ovT_f[:, :, :],
                                       ps_tt[:, :]
                                       .rearrange("p (k b) -> p k b", k=16))
                    lhs_o, NBO = ovT_f, B
                else:
                    lhs_o, NBO = ovT, BP

                out_sb = qsb.tile([NBO, HO], F32)
                for n0 in range(0, HO, 512):
                    nn = min(512, HO - n0)
                    wo_t = wop.tile([128, 16, 512], F32, tag="wo")
                    nc.sync.dma_start(
                        wo_t[:, :, :nn],
                        w_o[:, n0:n0 + nn]
                        .rearrange("(t p) n -> p t n", p=128))
                    ps_o = psoo.tile([NBO, 512], F32, tag="oproj")
                    for kt in range(16):
                        nc.tensor.matmul(ps_o[:, :nn], lhs_o[:, kt, :],
                                         wo_t[:, kt, :nn],
                                         start=(kt == 0), stop=(kt == 15))
                    nc.any.tensor_copy(out_sb[:, n0:n0 + nn], ps_o[:, :nn])
                nc.sync.dma_start(out[:, :], out_sb[:, :])

    nc.compile()
    return nc


# ----------------------------- host wrapper ------------------------------


def _prep_in_maps(inputs, S, n_cores, tp):
    hidden = np.asarray(inputs["hidden_states"], np.float32)
    pos = np.asarray(inputs["positions"], np.int32)
    w_qkv_a = np.asarray(inputs["w_qkv_a"], np.float32)
    q_a_norm_w = np.asarray(inputs["q_a_norm_w"], np.float32)
    w_q_b = np.asarray(inputs["w_q_b"], np.float32)
    kv_a_norm_w = np.asarray(inputs["kv_a_norm_w"], np.float32)
    w_kc = np.asarray(inputs["w_kc"], np.float32)
    w_vc = np.asarray(inputs["w_vc"], np.float32)
    w_o = np.asarray(inputs["w_o"], np.float32)
    cache_l = np.asarray(inputs["kv_cache_latent"], np.float32)
    cache_r = np.asarray(inputs["kv_cache_rope"], np.float32)

    # current-token cache update (host)
    latent = hidden @ w_qkv_a[:, QL:QL + KL]
    k_pe = hidden @ w_qkv_a[:, QL + KL:]
    latent_n = _rmsnorm_np(latent, kv_a_norm_w)
    k_pe_r = _rope_np(k_pe.astype(np.float32), pos)
    cache_l = cache_l.copy()
    cache_r = cache_r.copy()
    cache_l[:, -1, :] = latent_n
    cache_r[:, -1, :] = k_pe_r
    cacheT_l = np.ascontiguousarray(cache_l.transpose(0, 2, 1))
    cacheT_r = np.ascontiguousarray(cache_r.transpose(0, 2, 1))

    hiddenT = np.ascontiguousarray(hidden.T)
    w_qb_eff = np.ascontiguousarray(q_a_norm_w[:, None] * w_q_b)
    RT = _rope_RT(pos)
    w_qa_q = np.ascontiguousarray(w_qkv_a[:, :QL])

    in_maps = []
    for k in range(n_cores):
        b0 = k * BP
        if tp:
            k0 = k * (HID // n_cores)
            k1 = (k + 1) * (HID // n_cores)
            m = {
                "hT": np.ascontiguousarray(hiddenT[k0:k1, :]),
                "w_qa": np.ascontiguousarray(w_qa_q[k0:k1, :]),
                "w_o": np.ascontiguousarray(
                    w_o[:, k * (HID // n_cores):(k + 1) * (HID // n_cores)]),
            }
        else:
            m = {
                "hT": np.ascontiguousarray(hiddenT[:, b0:b0 + BP]),
                "w_qa": w_qa_q,
                "w_o": np.ascontiguousarray(w_o),
            }
        m.update({
            "w_qb": w_qb_eff,
            "w_kc": np.ascontiguousarray(w_kc),
            "w_vc": np.ascontiguousarray(w_vc),
            "cache_nat": np.ascontiguousarray(cache_l[b0:b0 + BP, :S, :]),
            "cacheT_l": np.ascontiguousarray(cacheT_l[b0:b0 + BP, :, :S]),
            "cacheT_r": np.ascontiguousarray(cacheT_r[b0:b0 + BP, :, :S]),
            "ropeRT": np.ascontiguousarray(RT[b0:b0 + BP]),
        })
        in_maps.append(m)
    return in_maps


def _unshard(results, tp):
    if tp:
        return np.concatenate([results[k]["out"] for k in range(N_CORES)],
                              axis=1)
    return np.concatenate([results[k]["out"] for k in range(N_CORES)], axis=0)


def run(inputs, S=4096, trace=False):
    key = (S, N_CORES, TP)
    if key not in _CACHE:
        _CACHE[key] = _build(S, N_CORES, TP)
    nc = _CACHE[key]
    in_maps = _prep_in_maps(inputs, S, N_CORES, TP)
    res = bass_utils.run_bass_kernel_spmd(
        nc, in_maps, core_ids=list(range(N_CORES)), trace=trace)
    return _unshard(res.results, TP), res


def kernel(**inputs) -> np.ndarray:
    out, _ = run(inputs)
    return out.astype(np.float32)



# revision 13
# speedup vs baseline: 2.5012x; 2.5012x over previous
"""DeepseekV2 MLA decode attention on 8 Trainium2 NeuronCores.

Strategy (single SPMD launch, identical program on all cores):

  - Everything large flows in bf16 (cache + weights); accumulation in fp32
    PSUM. All matmuls are oriented so the moving (output free) dimension is
    small (heads=16 / batch=4/32) with the 128-contraction on partitions.
  - Attention is batch-sharded: core k owns sequences 4k..4k+4. The latent
    cache streams ONCE per core in natural [s, c] layout; the score-side
    [c, s] layout is produced on-chip by PE transposes + PSUM->SBUF copies
    spread across the ACT/DVE/Pool engines. Context matmuls consume the
    natural tiles directly and accumulate ctx^T [c, (cc,b,h)] in PSUM over
    the whole sequence (flash-style, unnormalized; 1/rowsum folded in at
    the end via an outer-product broadcast multiply).
  - q path: w_qkv_a is K-sharded over hidden; each core produces partial
    q_a^T [QL, B]; ReduceScatter #1 sums partials and hands each core a
    QL/8 row slice for ALL batches. w_q_b is QL-sharded to match, with the
    q_a rmsnorm weight AND the per-head w_kc absorption folded in on the
    host (q_abs comes straight out of one GEMM). Partial q rows
    [B, 9216 + 1 sumsq column] go through ReduceScatter #2 which sums the
    QL-slice partials and hands each core its own 4 sequences. The rmsnorm
    1/rms scalar (from the sumsq column) is folded into the per-batch q
    transpose matmuls as a diagonal rhs.
  - rope(q_pe) as matmul with host-prepared per-batch rotation matrices.
  - w_o is column-sharded; ov rows are AllGathered, each core produces a
    640-column slice of the output transposed [640, B]; host re-transposes.
  - The current-token cache update (rmsnorm latent / roped k_pe at slot
    S-1) is applied on the host while building the bf16 cache layout (as
    in the original baseline).
"""

import sys

sys.path.insert(0, "/opt/trn_rl_repo")

import ml_dtypes
import numpy as np

import concourse.bacc as bacc
import concourse.mybir as mybir
import concourse.tile as tile
from concourse import bass_utils
from concourse.masks import make_identity

F32 = mybir.dt.float32
BF16 = mybir.dt.bfloat16
ADD = mybir.AluOpType.add
MULT = mybir.AluOpType.mult
BYPASS = mybir.AluOpType.bypass
EXP = mybir.ActivationFunctionType.Exp
SQRT = mybir.ActivationFunctionType.Sqrt
AXIS_X = mybir.AxisListType.X

B, HID, H = 32, 5120, 16
DN, DR, DV = 128, 64, 128
QL, KL = 1536, 512
BASE = 10000.0
EPS = 1e-6
SCALE = float((DN + DR) ** -0.5)

N_CORES = 8
BP = B // N_CORES          # sequences per core
KH = HID // N_CORES        # hidden slice per core (640)
QLS = QL // N_CORES        # q_lora slice per core (192)
NABS = H * KL              # absorbed q columns (8192)
NQB = NABS + H * DR        # folded q_b columns (9216)
HO = HID // N_CORES        # output columns per core (640)
TP = True

NPBF = ml_dtypes.bfloat16

_CACHE = {}


# ----------------------------- host math ---------------------------------


def _rmsnorm_np(x, w):
    ms = np.mean(x * x, axis=-1, keepdims=True, dtype=np.float32)
    return (x * (1.0 / np.sqrt(ms + EPS)) * w).astype(np.float32)


def _rope_np(x, pos):
    d = x.shape[-1]
    inv = (1.0 / (BASE ** (np.arange(0, d, 2, dtype=np.float32) / d))).astype(
        np.float32
    )
    fr = pos.astype(np.float32)[:, None] * inv
    cos, sin = np.cos(fr).astype(np.float32), np.sin(fr).astype(np.float32)
    out = np.empty_like(x)
    out[..., 0::2] = x[..., 0::2] * cos - x[..., 1::2] * sin
    out[..., 1::2] = x[..., 1::2] * cos + x[..., 0::2] * sin
    return out.astype(np.float32)


def _rope_RT(pos):
    """Per-batch transposed rotation matrices (lhsT for rope-as-matmul)."""
    inv = (1.0 / (BASE ** (np.arange(0, DR, 2, dtype=np.float32) / DR))).astype(
        np.float32
    )
    fr = pos.astype(np.float32)[:, None] * inv
    cos, sin = np.cos(fr).astype(np.float32), np.sin(fr).astype(np.float32)
    R = np.zeros((B, DR, DR), np.float32)
    j = np.arange(DR // 2)
    bi = np.arange(B)[:, None]
    R[bi, 2 * j, 2 * j] = cos
    R[bi, 2 * j, 2 * j + 1] = -sin
    R[bi, 2 * j + 1, 2 * j] = sin
    R[bi, 2 * j + 1, 2 * j + 1] = cos
    return np.ascontiguousarray(R.transpose(0, 2, 1))


# ----------------------------- device program ----------------------------


def _build(S, n_cores, tp=True, fake_coll=False, debug=False):
    nc = bacc.Bacc("TRN2", target_bir_lowering=False, debug=False,
                   enable_asserts=False, num_devices=n_cores)
    ST = S // 512
    rg = [list(range(n_cores))]

    hT = nc.dram_tensor("hT", [KH, B], BF16, kind="ExternalInput")
    w_qa = nc.dram_tensor("w_qa", [KH, QL], BF16, kind="ExternalInput")
    w_qbx = nc.dram_tensor("w_qbx", [QLS, NQB], BF16, kind="ExternalInput")
    ropeRT = nc.dram_tensor("ropeRT", [BP, DR, DR], BF16,
                            kind="ExternalInput")
    cache_nat = nc.dram_tensor("cache_nat", [BP, S, KL], BF16,
                               kind="ExternalInput")
    cacheT_r = nc.dram_tensor("cacheT_r", [BP, DR, S], BF16,
                              kind="ExternalInput")
    wvc_p = nc.dram_tensor("wvc_p", [128, H * 4 * DV], BF16,
                           kind="ExternalInput")
    wo_p = nc.dram_tensor("wo_p", [128, 16 * HO], BF16, kind="ExternalInput")
    out = nc.dram_tensor("out", [HO, B], F32, kind="ExternalOutput")
    if debug:
        d_qrow = nc.dram_tensor("d_qrow", [BP, NQB + 1], BF16,
                                kind="ExternalOutput")
        d_qabs = nc.dram_tensor("d_qabs", [128, 4 * H * BP], BF16,
                                kind="ExternalOutput")
        d_qpe = nc.dram_tensor("d_qpe", [DR, BP * H], BF16,
                               kind="ExternalOutput")
        d_e0 = nc.dram_tensor("d_e0", [128, 4 * H], BF16,
                              kind="ExternalOutput")
        d_sums = nc.dram_tensor("d_sums", [H, BP], F32,
                                kind="ExternalOutput")
        d_qat = nc.dram_tensor("d_qat", [128, 12 * B], BF16,
                               kind="ExternalOutput")
        d_qa = nc.dram_tensor("d_qa", [128, 2 * B], BF16,
                              kind="ExternalOutput")
        d_ctxn = nc.dram_tensor("d_ctxn", [128, 4 * BP * H], BF16,
                                kind="ExternalOutput")
        d_ov = nc.dram_tensor("d_ov", [BP, H * DV], BF16,
                              kind="ExternalOutput")

    with tile.TileContext(nc) as tc:
        with (
            tc.tile_pool(name="const", bufs=1) as cp,
            tc.tile_pool(name="qsb", bufs=1) as qsb,
            tc.tile_pool(name="dram", bufs=1, space="DRAM") as dramp,
            tc.tile_pool(name="wqa", bufs=5) as wqap,
            tc.tile_pool(name="nat", bufs=6) as natp,
            tc.tile_pool(name="trc", bufs=4) as trcp,
            tc.tile_pool(name="et", bufs=3) as etp,
            tc.tile_pool(name="ctr", bufs=2) as ctrp,
            tc.tile_pool(name="small", bufs=1) as smp,
        ):
            # ---------------- constants ----------------
            ident_f = cp.tile([128, 128], F32)
            make_identity(nc, ident_f[:, :])
            ident_bf = cp.tile([128, 128], BF16)
            nc.scalar.copy(ident_bf[:, :], ident_f[:, :])
            ones_col = cp.tile([128, 1], BF16)
            nc.any.memset(ones_col, 1.0)
            ones_f = cp.tile([128, 1], F32)
            nc.any.memset(ones_f, 1.0)
            ones16 = cp.tile([16, 128], BF16)
            nc.any.memset(ones16, 1.0)
            eps_t = cp.tile([128, 1], F32)
            nc.any.memset(eps_t, EPS)

            # ---------------- front loads (SP queue) ----------------
            hT_sb = cp.tile([128, 5, B], BF16)
            nc.sync.dma_start(hT_sb[:, :, :],
                              hT[:, :].rearrange("(t p) b -> p t b", p=128))
            wqa_sb = []
            for kt in range(5):
                wt = wqap.tile([128, QL], BF16, tag="wqa", name=f"wqa{kt}")
                nc.sync.dma_start(wt[:, :], w_qa[kt * 128:(kt + 1) * 128, :])
                wqa_sb.append(wt)
            wqbx_sb = qsb.tile([128, 2, NQB], BF16)
            nc.sync.dma_start(wqbx_sb[:, 0, :], w_qbx[0:128, :])
            nc.sync.dma_start(wqbx_sb[:64, 1, :], w_qbx[128:QLS, :])
            rt_sb = cp.tile([DR, BP, DR], BF16)
            nc.sync.dma_start(rt_sb[:, :, :],
                              ropeRT[:, :, :].rearrange("b k m -> k b m"))

            # DRAM scratch for collectives
            rs1_in = dramp.tile([QL, B], BF16)
            rs1_out = dramp.tile([QLS, B], BF16)
            rs2_in = dramp.tile([B, NQB + 1], BF16)
            rs2_out = dramp.tile([BP, NQB + 1], BF16)
            ag_in = dramp.tile([BP, H * DV], BF16)
            ag_out = dramp.tile([B, H * DV], BF16)

            qabsT = qsb.tile([128, 4, H, BP], BF16)   # [c, cc, h, b]
            qpeT = qsb.tile([DR, BP, H], BF16)        # [r, b, h]

            # ================= q path =================
            with (
                tc.tile_pool(name="psqa", bufs=1, space="PSUM") as psqa,
                tc.tile_pool(name="psqb", bufs=3, space="PSUM") as psqb,
                tc.tile_pool(name="psqc", bufs=2, space="PSUM") as psqc,
            ):
                # ---- qkv_a partial, transposed: qaT [QL, B] ----
                qa_ps = psqa.tile([128, 12 * B], F32, tag="qa")
                for cc in range(12):
                    for kt in range(5):
                        nc.tensor.matmul(
                            qa_ps[:, cc * B:(cc + 1) * B],
                            wqa_sb[kt][:, cc * 128:(cc + 1) * 128],
                            hT_sb[:, kt, :],
                            start=(kt == 0), stop=(kt == 4))
                qaT_sb = qsb.tile([128, 12, B], BF16)
                nc.scalar.copy(qaT_sb[:, :, :],
                               qa_ps[:, :].rearrange("p (t b) -> p t b", t=12))
                nc.scalar.dma_start(
                    rs1_in[:, :].rearrange("(t p) b -> p t b", p=128),
                    qaT_sb[:, :, :])

                # ---- RS1: sum partials over hidden, split QL 8-ways ----
                if fake_coll:
                    nc.gpsimd.dma_start(rs1_out[:, :], rs1_in[0:QLS, :])
                else:
                    nc.gpsimd.collective_compute(
                        "ReduceScatter", ADD, replica_groups=rg,
                        ins=[rs1_in.opt()], outs=[rs1_out.opt()])
                qa_sb = qsb.tile([128, 2, B], BF16)
                nc.scalar.dma_start(qa_sb[:, 0, :], rs1_out[0:128, :])
                nc.scalar.dma_start(qa_sb[:64, 1, :], rs1_out[128:QLS, :])

                if debug:
                    nc.scalar.dma_start(
                        d_qat[:, :],
                        qaT_sb[:, :, :].rearrange("p t b -> p (t b)"))
                    nc.scalar.dma_start(
                        d_qa[:, :],
                        qa_sb[:, :, :].rearrange("p t b -> p (t b)"))

                # ---- partial sumsq per batch (for rmsnorm) ----
                sq1 = smp.tile([128, B], F32, tag="sq1")
                nc.vector.tensor_tensor(sq1[:, :], qa_sb[:, 0, :],
                                        qa_sb[:, 0, :], MULT)
                sq2 = smp.tile([64, B], F32, tag="sq2")
                nc.vector.tensor_tensor(sq2[:, :], qa_sb[:64, 1, :],
                                        qa_sb[:64, 1, :], MULT)
                ssq_ps = psqc.tile([1, B], F32, tag="q", name="ssq")
                nc.tensor.matmul(ssq_ps[:1, :], ones_f[:, :1], sq1[:, :],
                                 start=True, stop=False)
                nc.tensor.matmul(ssq_ps[:1, :], ones_f[:64, :1], sq2[:, :],
                                 start=False, stop=True)
                ssq_sb = smp.tile([1, B], F32, tag="ssqsb")
                nc.vector.tensor_copy(ssq_sb[:, :], ssq_ps[:1, :])
                ssqT_ps = psqc.tile([B, 1], F32, tag="q", name="ssqT")
                nc.tensor.transpose(ssqT_ps[:B, :1], ssq_sb[:1, :B],
                                    ident_f[:1, :1])
                ssqT_sb = smp.tile([B, 1], BF16, tag="ssqTsb")
                nc.vector.tensor_copy(ssqT_sb[:, :], ssqT_ps[:B, :1])

                # ---- q_b on the QL slice (folded norm + absorb) ----
                qrows_sb = qsb.tile([B, NQB], BF16)
                for nb in range(18):
                    ps = psqb.tile([B, 512], F32, tag="qb", name=f"qb{nb}")
                    nc.tensor.matmul(ps[:, :], qa_sb[:, 0, :],
                                     wqbx_sb[:, 0, nb * 512:(nb + 1) * 512],
                                     start=True, stop=False)
                    nc.tensor.matmul(ps[:, :], qa_sb[:64, 1, :],
                                     wqbx_sb[:64, 1, nb * 512:(nb + 1) * 512],
                                     start=False, stop=True)
                    dst = qrows_sb[:, nb * 512:(nb + 1) * 512]
                    if nb % 2 == 0:
                        nc.scalar.copy(dst, ps[:, :])
                    else:
                        nc.vector.tensor_copy(dst, ps[:, :])
                nc.scalar.dma_start(rs2_in[:, 0:NQB], qrows_sb[:, :])
                nc.scalar.dma_start(rs2_in[:, NQB:NQB + 1], ssqT_sb[:, :])

                # ---- RS2: sum QL-slice partials, split batch 8-ways ----
                if fake_coll:
                    nc.gpsimd.dma_start(rs2_out[:, :], rs2_in[0:BP, :])
                else:
                    nc.gpsimd.collective_compute(
                        "ReduceScatter", ADD, replica_groups=rg,
                        ins=[rs2_in.opt()], outs=[rs2_out.opt()])
                qrow4 = qsb.tile([BP, NQB + 1], BF16)
                nc.scalar.dma_start(qrow4[:, :], rs2_out[:, :])

                # ---- rinv from the sumsq column; diag(rinv) in bf16 ----
                rms = smp.tile([BP, 1], F32, tag="rms")
                nc.scalar.activation(rms[:, :], qrow4[:, NQB:NQB + 1], SQRT,
                                     bias=eps_t[:BP, :1], scale=1.0 / QL)
                rinv = smp.tile([BP, 1], F32, tag="rinv")
                nc.vector.reciprocal(rinv[:, :], rms[:, :])
                diag4 = smp.tile([BP, BP], BF16, tag="diag4")
                nc.vector.tensor_scalar_mul(diag4[:, :], ident_bf[:BP, :BP],
                                            rinv[:, :1])

                # ---- q transposes with rinv folded in (diag rhs) ----
                # qabs_ps cols (cc, h, b); qpe_ps cols (h, b)
                qabs_ps = psqc.tile([128, 4 * H * BP], F32, tag="q", name="qabs")
                for h in range(H):
                    for cc in range(4):
                        c0 = (cc * H + h) * BP
                        nc.tensor.matmul(
                            qabs_ps[:, c0:c0 + BP],
                            qrow4[:, h * KL + cc * 128:h * KL + (cc + 1) * 128],
                            diag4[:, :], start=True, stop=True)
                nc.scalar.copy(
                    qabsT[:, :, :, :],
                    qabs_ps[:, :].rearrange("p (c h b) -> p c h b", c=4, h=H))
                qpe_ps = psqc.tile([DR, H * BP], F32, tag="q", name="qpe")
                for h in range(H):
                    nc.tensor.matmul(
                        qpe_ps[:DR, h * BP:(h + 1) * BP],
                        qrow4[:, NABS + h * DR:NABS + (h + 1) * DR],
                        diag4[:, :], start=True, stop=True)
                qpe_sb = qsb.tile([DR, H, BP], BF16)
                nc.vector.tensor_copy(
                    qpe_sb[:, :, :],
                    qpe_ps[:DR, :].rearrange("p (h b) -> p h b", h=H))

                if debug:
                    nc.scalar.dma_start(d_qrow[:, :], qrow4[:, :])
                    nc.scalar.dma_start(
                        d_qabs[:, :],
                        qabsT[:, :, :, :].rearrange("p c h b -> p (c h b)"))

                # ---- rope(q_pe): per-batch rotation matmul ----
                qrope_ps = psqc.tile([DR, BP * H], F32, tag="q", name="qrope")
                for b in range(BP):
                    nc.tensor.matmul(
                        qrope_ps[:DR, b * H:(b + 1) * H],
                        rt_sb[:, b, :], qpe_sb[:, :, b],
                        start=True, stop=True)
                nc.vector.tensor_copy(
                    qpeT[:, :, :],
                    qrope_ps[:DR, :].rearrange("p (b h) -> p b h", b=BP))
                if debug:
                    nc.scalar.dma_start(
                        d_qpe[:, :],
                        qpeT[:, :, :].rearrange("p b h -> p (b h)"))

            # ================= attention =================
            wvc_sb = qsb.tile([128, H, 4, DV], BF16)
            wo_sb = qsb.tile([128, 16, HO], BF16)
            with (
                tc.tile_pool(name="pstr", bufs=2, space="PSUM") as pstr,
                tc.tile_pool(name="pssc", bufs=1, space="PSUM") as pssc,
                tc.tile_pool(name="psctx", bufs=1, space="PSUM") as psctx,
                tc.tile_pool(name="pssum", bufs=1, space="PSUM") as pssum,
            ):
                # one ctx tile per cc chunk: accumulation groups sharing a
                # PSUM bank must not interleave, so each long-running group
                # gets its own bank; cols (b, h)
                ctx_ps = [psctx.tile([128, BP * H], F32, tag=f"ctx{cc}",
                                     name=f"ctx{cc}") for cc in range(4)]
                sums_ps = pssum.tile([H, BP], F32, tag="sums")

                for lb in range(BP):
                    ctr_sb = ctrp.tile([DR, S], BF16, tag="ctr")
                    nc.sync.dma_start(ctr_sb[:, :], cacheT_r[lb, :, :])
                    for st in range(ST):
                        s0 = st * 512
                        nt = natp.tile([128, 4, KL], BF16, tag="nat")
                        nc.sync.dma_start(
                            nt[:, :, :],
                            cache_nat[lb, s0:s0 + 512, :]
                            .rearrange("(i p) c -> p i c", p=128))
                        # transpose the tile on PE: [s, c] -> [c, s] chunks;
                        # two cc chunks per PSUM tile (full bank)
                        trc = []
                        for j in range(2):
                            tr_ps = pstr.tile([128, 1024], BF16, tag="tr",
                                              name=f"tr{lb}_{st}_{j}")
                            for cj in range(2):
                                cc = j * 2 + cj
                                for i in range(4):
                                    nc.tensor.transpose(
                                        tr_ps[:, cj * 512 + i * 128:
                                              cj * 512 + (i + 1) * 128],
                                        nt[:, i, cc * 128:(cc + 1) * 128],
                                        ident_bf[:, :])
                            tc_sb = trcp.tile([128, 1024], BF16, tag="trc",
                                              name=f"trc{lb}_{st}_{j}")
                            if j == 0:
                                nc.scalar.copy(tc_sb[:, :], tr_ps[:, :])
                            else:
                                nc.vector.tensor_copy(tc_sb[:, :], tr_ps[:, :])
                            trc.append(tc_sb)
                        # scores (transposed): sc [s, (i, h)]
                        sc = pssc.tile([128, 4 * H], F32, tag="sc",
                                       name=f"sc{lb}_{st}")
                        for i in range(4):
                            for cc in range(4):
                                nc.tensor.matmul(
                                    sc[:, i * H:(i + 1) * H],
                                    trc[cc // 2][:, (cc % 2) * 512 + i * 128:
                                                 (cc % 2) * 512 +
                                                 (i + 1) * 128],
                                    qabsT[:, cc, :, lb],
                                    start=(cc == 0), stop=False)
                            nc.tensor.matmul(
                                sc[:, i * H:(i + 1) * H],
                                ctr_sb[:, s0 + i * 128:s0 + (i + 1) * 128],
                                qpeT[:, lb, :], start=False, stop=True)
                        eT = etp.tile([128, 4 * H], BF16, tag="eT",
                                      name=f"eT{lb}_{st}")
                        nc.scalar.activation(eT[:, :], sc[:, :], EXP,
                                             scale=SCALE)
                        if debug and lb == 0 and st == 0:
                            nc.scalar.dma_start(d_e0[:, :], eT[:, :])
                        # context accumulation + row sums
                        for i in range(4):
                            for cc in range(4):
                                c0 = lb * H
                                nc.tensor.matmul(
                                    ctx_ps[cc][:, c0:c0 + H],
                                    nt[:, i, cc * 128:(cc + 1) * 128],
                                    eT[:, i * H:(i + 1) * H],
                                    start=(st == 0 and i == 0),
                                    stop=(st == ST - 1 and i == 3))
                            nc.tensor.matmul(
                                sums_ps[:H, lb:lb + 1],
                                eT[:, i * H:(i + 1) * H], ones_col[:, :1],
                                start=(st == 0 and i == 0),
                                stop=(st == ST - 1 and i == 3))
                    if lb == 2:
                        # value/output weights land mid-stream on the SP queue
                        nc.sync.dma_start(
                            wvc_sb[:, :, :, :],
                            wvc_p[:, :].rearrange("p (h c v) -> p h c v",
                                                  h=H, c=4))
                        nc.sync.dma_start(
                            wo_sb[:, :, :],
                            wo_p[:, :].rearrange("p (t n) -> p t n", t=16))

                # ---- 1/rowsum broadcast, ctx normalize ----
                rec_f = smp.tile([H, BP], F32, tag="rec")
                nc.vector.reciprocal(rec_f[:, :], sums_ps[:H, :])
                diag16 = smp.tile([H, BP, H], BF16, tag="diag16")
                for b in range(BP):
                    nc.vector.tensor_scalar_mul(diag16[:, b, :],
                                                ident_bf[:H, :H],
                                                rec_f[:, b:b + 1])
                bc_ps = pssc.tile([128, 4 * BP * H], F32, tag="sc", name="bc")
                for cc in range(4):
                    for b in range(BP):
                        c0 = (cc * BP + b) * H
                        nc.tensor.matmul(bc_ps[:, c0:c0 + H],
                                         ones16[:H, :], diag16[:, b, :],
                                         start=True, stop=True)
                bc_sb = smp.tile([128, 4 * BP * H], BF16, tag="bcsb")
                nc.scalar.copy(bc_sb[:, :], bc_ps[:, :])
                ctxn_sb = qsb.tile([128, 4, BP, H], BF16)
                for cc in range(4):
                    nc.vector.tensor_tensor(
                        ctxn_sb[:, cc, :, :],
                        ctx_ps[cc][:, :].rearrange("p (b h) -> p b h", b=BP),
                        bc_sb[:, cc * BP * H:(cc + 1) * BP * H]
                        .rearrange("p (b h) -> p b h", b=BP),
                        MULT)

                if debug:
                    ds = smp.tile([H, BP], F32, tag="dsums")
                    nc.vector.tensor_copy(ds[:, :], sums_ps[:H, :])
                    nc.scalar.dma_start(d_sums[:, :], ds[:, :])
                    nc.scalar.dma_start(
                        d_ctxn[:, :],
                        ctxn_sb[:, :, :, :].rearrange("p c b h -> p (c b h)"))

                # ---- un-absorb values: ovT [v, (h, b)] ----
                psv = pssc.tile([128, H * BP], F32, tag="sc", name="psv")
                for h in range(H):
                    for cc in range(4):
                        nc.tensor.matmul(
                            psv[:, h * BP:(h + 1) * BP],
                            wvc_sb[:, h, cc, :], ctxn_sb[:, cc, :, h],
                            start=(cc == 0), stop=(cc == 3))
                ovT = qsb.tile([128, H, BP], BF16)
                nc.scalar.copy(ovT[:, :, :],
                               psv[:, :].rearrange("p (h b) -> p h b", h=H))

            # ================= output =================
            with (
                tc.tile_pool(name="psor", bufs=2, space="PSUM") as psor,
                tc.tile_pool(name="psot", bufs=1, space="PSUM") as psot,
                tc.tile_pool(name="psoo", bufs=1, space="PSUM") as psoo,
            ):
                # ovT -> rows [4, 2048] -> AllGather -> [32, 2048] -> T
                ov_rows = qsb.tile([BP, H * DV], BF16)
                for g in range(4):
                    rp = psor.tile([BP, 4 * DV], BF16, tag="rows",
                                   name=f"rows{g}")
                    for hh in range(4):
                        h = g * 4 + hh
                        nc.tensor.transpose(rp[:BP, hh * DV:(hh + 1) * DV],
                                            ovT[:, h, :], ident_bf[:, :])
                    dst = ov_rows[:, g * 4 * DV:(g + 1) * 4 * DV]
                    if g % 2 == 0:
                        nc.scalar.copy(dst, rp[:BP, :])
                    else:
                        nc.vector.tensor_copy(dst, rp[:BP, :])
                if debug:
                    nc.scalar.dma_start(d_ov[:, :], ov_rows[:, :])
                nc.scalar.dma_start(ag_in[:, :], ov_rows[:, :])
                if fake_coll:
                    nc.gpsimd.dma_start(ag_out[0:BP, :], ag_in[:, :])
                else:
                    nc.gpsimd.collective_compute(
                        "AllGather", BYPASS, replica_groups=rg,
                        ins=[ag_in.opt()], outs=[ag_out.opt()])
                ov32_sb = qsb.tile([B, H * DV], BF16)
                nc.scalar.dma_start(ov32_sb[:, :], ag_out[:, :])
                of_ps = psot.tile([128, 16 * B], BF16, tag="ovT")
                for t in range(16):
                    nc.tensor.transpose(of_ps[:, t * B:(t + 1) * B],
                                        ov32_sb[:B, t * 128:(t + 1) * 128],
                                        ident_bf[:B, :B])
                ovT_f = qsb.tile([128, 16, B], BF16)
                nc.scalar.copy(ovT_f[:, :, :],
                               of_ps[:, :].rearrange("p (t b) -> p t b", t=16))
                po = psoo.tile([128, 5 * B], F32, tag="oproj")
                for c5 in range(5):
                    for t in range(16):
                        nc.tensor.matmul(
                            po[:, c5 * B:(c5 + 1) * B],
                            wo_sb[:, t, c5 * 128:(c5 + 1) * 128],
                            ovT_f[:, t, :],
                            start=(t == 0), stop=(t == 15))
                out_sb = qsb.tile([128, 5, B], F32)
                nc.scalar.copy(out_sb[:, :, :],
                               po[:, :].rearrange("p (t b) -> p t b", t=5))
                nc.scalar.dma_start(
                    out[:, :].rearrange("(t p) b -> p t b", p=128),
                    out_sb[:, :, :])

    nc.compile()
    return nc


# ----------------------------- host wrapper ------------------------------


def _prep_in_maps(inputs, S, n_cores, tp=True):
    hidden = np.asarray(inputs["hidden_states"], np.float32)
    pos = np.asarray(inputs["positions"], np.int32)
    w_qkv_a = np.asarray(inputs["w_qkv_a"], np.float32)
    q_a_norm_w = np.asarray(inputs["q_a_norm_w"], np.float32)
    w_q_b = np.asarray(inputs["w_q_b"], np.float32)
    kv_a_norm_w = np.asarray(inputs["kv_a_norm_w"], np.float32)
    w_kc = np.asarray(inputs["w_kc"], np.float32)
    w_vc = np.asarray(inputs["w_vc"], np.float32)
    w_o = np.asarray(inputs["w_o"], np.float32)
    cache_l = np.asarray(inputs["kv_cache_latent"], np.float32)
    cache_r = np.asarray(inputs["kv_cache_rope"], np.float32)

    # current-token cache update (host, as in the original baseline)
    latent = hidden @ w_qkv_a[:, QL:QL + KL]
    k_pe = hidden @ w_qkv_a[:, QL + KL:]
    latent_n = _rmsnorm_np(latent, kv_a_norm_w)
    k_pe_r = _rope_np(k_pe.astype(np.float32), pos)
    cache_l = cache_l.copy()
    cache_r = cache_r.copy()
    cache_l[:, -1, :] = latent_n
    cache_r[:, -1, :] = k_pe_r
    cache_nat = cache_l[:, :S, :].astype(NPBF)
    cacheT_r = np.ascontiguousarray(
        cache_r[:, :S, :].transpose(0, 2, 1)).astype(NPBF)

    hiddenT = np.ascontiguousarray(hidden.T).astype(NPBF)
    RT = _rope_RT(pos).astype(NPBF)

    # folded q_b: rmsnorm weight + per-head w_kc absorption
    w_qb_n = q_a_norm_w[:, None] * w_q_b                  # [QL, H*(DN+DR)]
    w3 = w_qb_n.reshape(QL, H, DN + DR)
    wabs = np.einsum("qhd,hdc->qhc", w3[:, :, :DN], w_kc)  # [QL, H, KL]
    w_qbx = np.concatenate(
        [wabs.reshape(QL, H * KL), w3[:, :, DN:].reshape(QL, H * DR)],
        axis=1).astype(NPBF)                               # [QL, 9216]

    w_qa_q = np.ascontiguousarray(w_qkv_a[:, :QL]).astype(NPBF)

    # packed value / output weights
    wvc_p = np.ascontiguousarray(
        w_vc.reshape(H, 4, 128, DV).transpose(2, 0, 1, 3)
        .reshape(128, H * 4 * DV)).astype(NPBF)
    wo3 = w_o.reshape(16, 128, HID)

    in_maps = []
    for k in range(n_cores):
        b0 = k * BP
        m = {
            "hT": np.ascontiguousarray(hiddenT[k * KH:(k + 1) * KH, :]),
            "w_qa": np.ascontiguousarray(w_qa_q[k * KH:(k + 1) * KH, :]),
            "w_qbx": np.ascontiguousarray(w_qbx[k * QLS:(k + 1) * QLS, :]),
            "ropeRT": np.ascontiguousarray(RT[b0:b0 + BP]),
            "cache_nat": np.ascontiguousarray(cache_nat[b0:b0 + BP]),
            "cacheT_r": np.ascontiguousarray(cacheT_r[b0:b0 + BP]),
            "wvc_p": wvc_p,
            "wo_p": np.ascontiguousarray(
                wo3[:, :, k * HO:(k + 1) * HO].transpose(1, 0, 2)
                .reshape(128, 16 * HO)).astype(NPBF),
        }
        in_maps.append(m)
    return in_maps


def _unshard(results, tp=True):
    return np.concatenate(
        [results[k]["out"].T for k in range(N_CORES)], axis=1)


def run(inputs, S=4096, trace=False):
    key = (S, N_CORES, TP)
    if key not in _CACHE:
        _CACHE[key] = _build(S, N_CORES, TP)
    nc = _CACHE[key]
    in_maps = _prep_in_maps(inputs, S, N_CORES, TP)
    res = bass_utils.run_bass_kernel_spmd(
        nc, in_maps, core_ids=list(range(N_CORES)), trace=trace)
    return _unshard(res.results, TP), res


def kernel(**inputs) -> np.ndarray:
    out, _ = run(inputs)
    return out.astype(np.float32)


# revision 42
# speedup vs baseline: 2.8846x; 1.1533x over previous
"""DeepseekV2 MLA decode attention on 8 Trainium2 NeuronCores.

Strategy (single SPMD launch, identical program on all cores):

  - Everything large flows in bf16 (cache + weights); accumulation in fp32
    PSUM. All matmuls are oriented so the moving (output free) dimension is
    small (heads=16 / batch=4/32) with the 128-contraction on partitions.
  - Attention is batch-sharded: core k owns sequences 4k..4k+4. The latent
    cache streams ONCE per core in natural [s, c] layout; the score-side
    [c, s] layout is produced on-chip by PE transposes + PSUM->SBUF copies
    spread across the ACT/DVE/Pool engines. Context matmuls consume the
    natural tiles directly and accumulate ctx^T [c, (cc,b,h)] in PSUM over
    the whole sequence (flash-style, unnormalized; 1/rowsum folded in at
    the end via an outer-product broadcast multiply).
  - q path: w_qkv_a is K-sharded over hidden; each core produces partial
    q_a^T [QL, B]; ReduceScatter #1 sums partials and hands each core a
    QL/8 row slice for ALL batches. w_q_b is QL-sharded to match, with the
    q_a rmsnorm weight AND the per-head w_kc absorption folded in on the
    host (q_abs comes straight out of one GEMM). Partial q rows
    [B, 9216 + 1 sumsq column] go through ReduceScatter #2 which sums the
    QL-slice partials and hands each core its own 4 sequences. The rmsnorm
    1/rms scalar (from the sumsq column) is folded into the per-batch q
    transpose matmuls as a diagonal rhs.
  - rope(q_pe) as matmul with host-prepared per-batch rotation matrices.
  - w_o is column-sharded; ov rows are AllGathered, each core produces a
    640-column slice of the output transposed [640, B]; host re-transposes.
  - The current-token cache update (rmsnorm latent / roped k_pe at slot
    S-1) is applied on the host while building the bf16 cache layout (as
    in the original baseline).
"""

import sys

sys.path.insert(0, "/opt/trn_rl_repo")

import ml_dtypes
import numpy as np

import concourse.bacc as bacc
import concourse.mybir as mybir
import concourse.tile as tile
from concourse import bass_utils
from concourse.masks import make_identity

F32 = mybir.dt.float32
BF16 = mybir.dt.bfloat16
ADD = mybir.AluOpType.add
MULT = mybir.AluOpType.mult
BYPASS = mybir.AluOpType.bypass
EXP = mybir.ActivationFunctionType.Exp
SQRT = mybir.ActivationFunctionType.Sqrt
AXIS_X = mybir.AxisListType.X

B, HID, H = 32, 5120, 16
DN, DR, DV = 128, 64, 128
QL, KL = 1536, 512
BASE = 10000.0
EPS = 1e-6
SCALE = float((DN + DR) ** -0.5)

N_CORES = 8
BP = B // N_CORES          # sequences per core
KH = HID // N_CORES        # hidden slice per core (640)
QLS = QL // N_CORES        # q_lora slice per core (192)
NABS = H * KL              # absorbed q columns (8192)
NQB = NABS + H * DR        # folded q_b columns (9216)
HO = HID // N_CORES        # output columns per core (640)
TP = True

NPBF = ml_dtypes.bfloat16

_CACHE = {}


# ----------------------------- host math ---------------------------------


def _rmsnorm_np(x, w):
    ms = np.mean(x * x, axis=-1, keepdims=True, dtype=np.float32)
    return (x * (1.0 / np.sqrt(ms + EPS)) * w).astype(np.float32)


def _rope_np(x, pos):
    d = x.shape[-1]
    inv = (1.0 / (BASE ** (np.arange(0, d, 2, dtype=np.float32) / d))).astype(
        np.float32
    )
    fr = pos.astype(np.float32)[:, None] * inv
    cos, sin = np.cos(fr).astype(np.float32), np.sin(fr).astype(np.float32)
    out = np.empty_like(x)
    out[..., 0::2] = x[..., 0::2] * cos - x[..., 1::2] * sin
    out[..., 1::2] = x[..., 1::2] * cos + x[..., 0::2] * sin
    return out.astype(np.float32)


def _rope_RT(pos):
    """Per-batch transposed rotation matrices (lhsT for rope-as-matmul)."""
    inv = (1.0 / (BASE ** (np.arange(0, DR, 2, dtype=np.float32) / DR))).astype(
        np.float32
    )
    fr = pos.astype(np.float32)[:, None] * inv
    cos, sin = np.cos(fr).astype(np.float32), np.sin(fr).astype(np.float32)
    R = np.zeros((B, DR, DR), np.float32)
    j = np.arange(DR // 2)
    bi = np.arange(B)[:, None]
    R[bi, 2 * j, 2 * j] = cos
    R[bi, 2 * j, 2 * j + 1] = -sin
    R[bi, 2 * j + 1, 2 * j] = sin
    R[bi, 2 * j + 1, 2 * j + 1] = cos
    return np.ascontiguousarray(R.transpose(0, 2, 1))


# ----------------------------- device program ----------------------------


def _build(S, n_cores, tp=True, fake_coll=False, debug=False,
           D1=2, D2=4, NATB=10, TRCB=6, COPYMODE=0, WARM=0, VCHAIN=1,
           PSTRB=4, SCB=3, VCB=1, WVCK=2, CTRQ=0, ETB=4):
    nc = bacc.Bacc("TRN2", target_bir_lowering=False, debug=False,
                   enable_asserts=False, num_devices=n_cores)
    ST = S // 512
    rg = [list(range(n_cores))]

    hT = nc.dram_tensor("hT", [KH, B], BF16, kind="ExternalInput")
    w_qa = nc.dram_tensor("w_qa", [KH, QL], BF16, kind="ExternalInput")
    w_qbx = nc.dram_tensor("w_qbx", [QLS, NQB], BF16, kind="ExternalInput")
    ropeRT = nc.dram_tensor("ropeRT", [BP, DR, DR], BF16,
                            kind="ExternalInput")
    cache_nat = nc.dram_tensor("cache_nat", [BP, S, KL], BF16,
                               kind="ExternalInput")
    cacheT_r = nc.dram_tensor("cacheT_r", [BP, DR, S], BF16,
                              kind="ExternalInput")
    wvc_p = nc.dram_tensor("wvc_p", [128, H * 4 * DV], BF16,
                           kind="ExternalInput")
    wo_p = nc.dram_tensor("wo_p", [128, 16 * HO], BF16, kind="ExternalInput")
    out = nc.dram_tensor("out", [HO, B], BF16, kind="ExternalOutput")
    if debug:
        d_qrow = nc.dram_tensor("d_qrow", [BP, NQB + 1], BF16,
                                kind="ExternalOutput")
        d_qabs = nc.dram_tensor("d_qabs", [128, 4 * H * BP], BF16,
                                kind="ExternalOutput")
        d_qpe = nc.dram_tensor("d_qpe", [DR, BP * H], BF16,
                               kind="ExternalOutput")
        d_e0 = nc.dram_tensor("d_e0", [128, 4 * H], BF16,
                              kind="ExternalOutput")
        d_sums = nc.dram_tensor("d_sums", [H, BP], F32,
                                kind="ExternalOutput")
        d_qat = nc.dram_tensor("d_qat", [128, 12 * B], BF16,
                               kind="ExternalOutput")
        d_qa = nc.dram_tensor("d_qa", [128, 2 * B], BF16,
                              kind="ExternalOutput")
        d_ctxn = nc.dram_tensor("d_ctxn", [128, 4 * BP * H], BF16,
                                kind="ExternalOutput")
        d_ov = nc.dram_tensor("d_ov", [BP, H * DV], BF16,
                              kind="ExternalOutput")

    with tile.TileContext(nc) as tc:
        with (
            tc.tile_pool(name="const", bufs=1) as cp,
            tc.tile_pool(name="qsb", bufs=1) as qsb,
            tc.tile_pool(name="nat", bufs=NATB) as natp,
            tc.tile_pool(name="trc", bufs=TRCB) as trcp,
            tc.tile_pool(name="et", bufs=ETB) as etp,
            tc.tile_pool(name="ctr", bufs=2) as ctrp,
            tc.tile_pool(name="dram", bufs=1, space="DRAM") as dramp,
            tc.tile_pool(name="small", bufs=1) as smp,
        ):
            # ---------------- constants ----------------
            ident_f = cp.tile([128, 128], F32)
            make_identity(nc, ident_f[:, :])
            ident_bf = cp.tile([128, 128], BF16)
            nc.scalar.copy(ident_bf[:, :], ident_f[:, :])
            ones_col = cp.tile([128, 1], BF16)
            nc.any.memset(ones_col, 1.0)
            ones_f = cp.tile([128, 1], F32)
            nc.any.memset(ones_f, 1.0)
            ones16 = cp.tile([16, 128], BF16)
            nc.any.memset(ones16, 1.0)
            eps_t = cp.tile([128, 1], F32)
            nc.any.memset(eps_t, EPS)

            # ---------------- front loads (SP queue) ----------------
            rt_sb = cp.tile([DR, BP, DR], BF16)
            nc.sync.dma_start(rt_sb[:, :, :],
                              ropeRT[:, :, :].rearrange("b k m -> k b m"))

            # DRAM scratch for collectives
            rs1_in = dramp.tile([QL, B], BF16)
            rs1_out = dramp.tile([QLS, B], BF16)
            rs2_in = dramp.tile([B, NQB + 1], BF16)
            rs2_out = dramp.tile([BP, NQB + 1], BF16)
            ag_in = dramp.tile([BP, H * DV], BF16)
            ag_out = dramp.tile([B, H * DV], BF16)

            qabsT = qsb.tile([128, 4, H, BP], BF16)   # [c, cc, h, b]
            qpeT = qsb.tile([DR, BP, H], BF16)        # [r, b, h]

            # ================= q path =================
            with (
                tc.tile_pool(name="psqa", bufs=1, space="PSUM") as psqa,
                tc.tile_pool(name="psqb", bufs=3, space="PSUM") as psqb,
                tc.tile_pool(name="psqc", bufs=2, space="PSUM") as psqc,
                tc.tile_pool(name="qtmp", bufs=1) as qtp,
                tc.tile_pool(name="wqa", bufs=5) as wqap,
            ):
                hT_sb = qtp.tile([128, 5, B], BF16)
                nc.sync.dma_start(hT_sb[:, :, :],
                                  hT[:, :].rearrange("(t p) b -> p t b",
                                                     p=128))
                wqa_sb = []
                for kt in range(5):
                    wt = wqap.tile([128, QL], BF16, tag="wqa",
                                   name=f"wqa{kt}")
                    nc.sync.dma_start(wt[:, :],
                                      w_qa[kt * 128:(kt + 1) * 128, :])
                    wqa_sb.append(wt)
                wqbx_sb = qtp.tile([128, 2, NQB], BF16)
                nc.sync.dma_start(wqbx_sb[:, 0, :], w_qbx[0:128, :])
                nc.sync.dma_start(wqbx_sb[:64, 1, :], w_qbx[128:QLS, :])
                # ---- qkv_a partial, transposed: qaT [QL, B] ----
                qa_ps = psqa.tile([128, 12 * B], F32, tag="qa")
                for cc in range(12):
                    for kt in range(5):
                        nc.tensor.matmul(
                            qa_ps[:, cc * B:(cc + 1) * B],
                            wqa_sb[kt][:, cc * 128:(cc + 1) * 128],
                            hT_sb[:, kt, :],
                            start=(kt == 0), stop=(kt == 4))
                qaT_sb = qtp.tile([128, 12, B], BF16)
                nc.scalar.copy(qaT_sb[:, :, :],
                               qa_ps[:, :].rearrange("p (t b) -> p t b", t=12))
                nc.scalar.dma_start(
                    rs1_in[:, :].rearrange("(t p) b -> p t b", p=128),
                    qaT_sb[:, :, :])

                # ---- RS1: sum partials over hidden, split QL 8-ways ----
                if fake_coll:
                    nc.gpsimd.dma_start(rs1_out[:, :], rs1_in[0:QLS, :])
                else:
                    nc.gpsimd.collective_compute(
                        "ReduceScatter", ADD, replica_groups=rg,
                        ins=[rs1_in.opt()], outs=[rs1_out.opt()])
                qa_sb = qtp.tile([128, 2, B], BF16)
                nc.scalar.dma_start(qa_sb[:, 0, :], rs1_out[0:128, :])
                nc.scalar.dma_start(qa_sb[:64, 1, :], rs1_out[128:QLS, :])

                if debug:
                    nc.scalar.dma_start(
                        d_qat[:, :],
                        qaT_sb[:, :, :].rearrange("p t b -> p (t b)"))
                    nc.scalar.dma_start(
                        d_qa[:, :],
                        qa_sb[:, :, :].rearrange("p t b -> p (t b)"))

                # ---- partial sumsq per batch (for rmsnorm) ----
                sq1 = smp.tile([128, B], F32, tag="sq1")
                nc.vector.tensor_tensor(sq1[:, :], qa_sb[:, 0, :],
                                        qa_sb[:, 0, :], MULT)
                sq2 = smp.tile([64, B], F32, tag="sq2")
                nc.vector.tensor_tensor(sq2[:, :], qa_sb[:64, 1, :],
                                        qa_sb[:64, 1, :], MULT)
                ssq_ps = psqc.tile([1, B], F32, tag="q", name="ssq")
                nc.tensor.matmul(ssq_ps[:1, :], ones_f[:, :1], sq1[:, :],
                                 start=True, stop=False)
                nc.tensor.matmul(ssq_ps[:1, :], ones_f[:64, :1], sq2[:, :],
                                 start=False, stop=True)
                ssq_sb = smp.tile([1, B], F32, tag="ssqsb")
                nc.vector.tensor_copy(ssq_sb[:, :], ssq_ps[:1, :])
                ssqT_ps = psqc.tile([B, 1], F32, tag="q", name="ssqT")
                nc.tensor.transpose(ssqT_ps[:B, :1], ssq_sb[:1, :B],
                                    ident_f[:1, :1])
                ssqT_sb = smp.tile([B, 1], BF16, tag="ssqTsb")
                nc.vector.tensor_copy(ssqT_sb[:, :], ssqT_ps[:B, :1])

                # ---- q_b on the QL slice (folded norm + absorb) ----
                qrows_sb = qtp.tile([B, NQB], BF16)
                for nb in range(18):
                    ps = psqb.tile([B, 512], F32, tag="qb", name=f"qb{nb}")
                    nc.tensor.matmul(ps[:, :], qa_sb[:, 0, :],
                                     wqbx_sb[:, 0, nb * 512:(nb + 1) * 512],
                                     start=True, stop=False)
                    nc.tensor.matmul(ps[:, :], qa_sb[:64, 1, :],
                                     wqbx_sb[:64, 1, nb * 512:(nb + 1) * 512],
                                     start=False, stop=True)
                    dst = qrows_sb[:, nb * 512:(nb + 1) * 512]
                    if nb % 2 == 0:
                        nc.scalar.copy(dst, ps[:, :])
                    else:
                        nc.vector.tensor_copy(dst, ps[:, :])
                nc.scalar.dma_start(rs2_in[:, 0:NQB], qrows_sb[:, :])
                nc.scalar.dma_start(rs2_in[:, NQB:NQB + 1], ssqT_sb[:, :])

                # ---- RS2: sum QL-slice partials, split batch 8-ways ----
                if fake_coll:
                    nc.gpsimd.dma_start(rs2_out[:, :], rs2_in[0:BP, :])
                else:
                    nc.gpsimd.collective_compute(
                        "ReduceScatter", ADD, replica_groups=rg,
                        ins=[rs2_in.opt()], outs=[rs2_out.opt()])
                qrow4 = qtp.tile([BP, NQB + 1], BF16)
                nc.scalar.dma_start(qrow4[:, :], rs2_out[:, :])

                # ---- PE warm-up: keep the tensor engine busy through
                # the RS2 wait so it enters attention at full clock ----
                for w in range(WARM):
                    wp_ps = psqc.tile([128, 128], BF16, tag="q",
                                      name=f"warm{w}")
                    nc.tensor.transpose(wp_ps[:, :], ident_bf[:, :],
                                        ident_bf[:, :])

                # ---- rinv from the sumsq column; diag(rinv) in bf16 ----
                rms = smp.tile([BP, 1], F32, tag="rms")
                nc.scalar.activation(rms[:, :], qrow4[:, NQB:NQB + 1], SQRT,
                                     bias=eps_t[:BP, :1], scale=1.0 / QL)
                rinv = smp.tile([BP, 1], F32, tag="rinv")
                nc.vector.reciprocal(rinv[:, :], rms[:, :])
                diag4 = smp.tile([BP, BP], BF16, tag="diag4")
                nc.vector.tensor_scalar_mul(diag4[:, :], ident_bf[:BP, :BP],
                                            rinv[:, :1])

                # ---- q transposes with rinv folded in (diag rhs) ----
                # qabs_ps cols (cc, h, b); qpe_ps cols (h, b)
                qabs_ps = psqc.tile([128, 4 * H * BP], F32, tag="q", name="qabs")
                for h in range(H):
                    for cc in range(4):
                        c0 = (cc * H + h) * BP
                        nc.tensor.matmul(
                            qabs_ps[:, c0:c0 + BP],
                            qrow4[:, h * KL + cc * 128:h * KL + (cc + 1) * 128],
                            diag4[:, :], start=True, stop=True)
                nc.scalar.copy(
                    qabsT[:, :, :, :],
                    qabs_ps[:, :].rearrange("p (c h b) -> p c h b", c=4, h=H))
                qpe_ps = psqc.tile([DR, H * BP], F32, tag="q", name="qpe")
                for h in range(H):
                    nc.tensor.matmul(
                        qpe_ps[:DR, h * BP:(h + 1) * BP],
                        qrow4[:, NABS + h * DR:NABS + (h + 1) * DR],
                        diag4[:, :], start=True, stop=True)
                qpe_sb = qtp.tile([DR, H, BP], BF16)
                nc.vector.tensor_copy(
                    qpe_sb[:, :, :],
                    qpe_ps[:DR, :].rearrange("p (h b) -> p h b", h=H))

                if debug:
                    nc.scalar.dma_start(d_qrow[:, :], qrow4[:, :])
                    nc.scalar.dma_start(
                        d_qabs[:, :],
                        qabsT[:, :, :, :].rearrange("p c h b -> p (c h b)"))

                # ---- rope(q_pe): per-batch rotation matmul ----
                qrope_ps = psqc.tile([DR, BP * H], F32, tag="q", name="qrope")
                for b in range(BP):
                    nc.tensor.matmul(
                        qrope_ps[:DR, b * H:(b + 1) * H],
                        rt_sb[:, b, :], qpe_sb[:, :, b],
                        start=True, stop=True)
                nc.vector.tensor_copy(
                    qpeT[:, :, :],
                    qrope_ps[:DR, :].rearrange("p (b h) -> p b h", b=BP))
                if debug:
                    nc.scalar.dma_start(
                        d_qpe[:, :],
                        qpeT[:, :, :].rearrange("p b h -> p (b h)"))

            # ================= attention =================
            wvc_sb = qsb.tile([128, H, 4, DV], BF16)
            wo_sb = qsb.tile([128, 16, HO], BF16)
            ctxn_sb = qsb.tile([128, 4, BP, H], BF16)
            ovT = qsb.tile([128, H, BP], BF16)
            with (
                tc.tile_pool(name="pstr", bufs=PSTRB, space="PSUM") as pstr,
                tc.tile_pool(name="pssc", bufs=SCB, space="PSUM") as pssc,
                tc.tile_pool(name="psctx", bufs=1, space="PSUM") as psctx,
            ):
                # ctx (cols (cc, b, h)) + row sums (cols 256..260) share one
                # pre-zeroed PSUM bank; every matmul accumulates with
                # start=False (no interleaved-group hazard without starts)
                ctx_ps = psctx.tile([128, 4 * BP * H + BP], F32, tag="ctx")
                sums_ps = ctx_ps[:H, 4 * BP * H:]
                nc.vector.memset(ctx_ps[:, :], 0.0)

                # software-pipelined: PE transposes tile k+1 while tile
                # k's PSUM->SBUF copies are in flight, then does tile k's
                # score/context matmuls.
                steps = [(lb, st) for lb in range(BP) for st in range(ST)]
                ctr_tiles = {}
                prev = None

                def load_stage(k):
                    lb, st = steps[k]
                    s0 = st * 512
                    if st == 0:
                        ct = ctrp.tile([DR, S], BF16, tag="ctr",
                                       name=f"ctr{lb}")
                        if CTRQ:
                            nc.gpsimd.dma_start(ct[:, :], cacheT_r[lb, :, :])
                        else:
                            nc.sync.dma_start(ct[:, :], cacheT_r[lb, :, :])
                        ctr_tiles[lb] = ct
                    nt = natp.tile([128, 4, KL], BF16, tag="nat",
                                   name=f"nat{lb}_{st}")
                    nc.sync.dma_start(
                        nt[:, :, :],
                        cache_nat[lb, s0:s0 + 512, :]
                        .rearrange("(i p) c -> p i c", p=128))
                    if k == WVCK:
                        # value weights early: per-lb value chains start at
                        # the end of lb=0's stream
                        nc.sync.dma_start(
                            wvc_sb[:, :, :, :],
                            wvc_p[:, :].rearrange("p (h c v) -> p h c v",
                                                  h=H, c=4))
                    if k == BP * ST - 9:
                        nc.sync.dma_start(
                            wo_sb[:, :, :],
                            wo_p[:, :].rearrange("p (t n) -> p t n", t=16))
                    # transpose on PE: [s, c] -> [c, s], 2 cc chunks per bank
                    trc = []
                    for j in range(2):
                        tr_ps = pstr.tile([128, 1024], BF16, tag="tr",
                                          name=f"tr{lb}_{st}_{j}")
                        for cj in range(2):
                            cc = j * 2 + cj
                            for i in range(4):
                                nc.tensor.transpose(
                                    tr_ps[:, cj * 512 + i * 128:
                                          cj * 512 + (i + 1) * 128],
                                    nt[:, i, cc * 128:(cc + 1) * 128],
                                    ident_bf[:, :])
                        tc_sb = trcp.tile([128, 1024], BF16, tag="trc",
                                          name=f"trc{lb}_{st}_{j}")
                        if COPYMODE == 1 or not (j == 0 and st % 2 == 0):
                            nc.vector.tensor_copy(tc_sb[:, :], tr_ps[:, :])
                        else:
                            nc.scalar.copy(tc_sb[:, :], tr_ps[:, :])
                        trc.append(tc_sb)
                    return (lb, st, nt, trc)

                def score_stage(state):
                    lb, st, nt, trc = state
                    s0 = st * 512
                    ctr_sb = ctr_tiles[lb]
                    sc = pssc.tile([128, 4 * H], F32, tag="sc",
                                   name=f"sc{lb}_{st}")
                    for i in range(4):
                        for cc in range(4):
                            nc.tensor.matmul(
                                sc[:, i * H:(i + 1) * H],
                                trc[cc // 2][:, (cc % 2) * 512 + i * 128:
                                             (cc % 2) * 512 + (i + 1) * 128],
                                qabsT[:, cc, :, lb],
                                start=(cc == 0), stop=False)
                        nc.tensor.matmul(
                            sc[:, i * H:(i + 1) * H],
                            ctr_sb[:, s0 + i * 128:s0 + (i + 1) * 128],
                            qpeT[:, lb, :], start=False, stop=True)
                    eT = etp.tile([128, 4 * H], BF16, tag="eT",
                                  name=f"eT{lb}_{st}")
                    nc.scalar.activation(eT[:, :], sc[:, :], EXP,
                                         scale=SCALE)
                    if debug and lb == 0 and st == 0:
                        nc.scalar.dma_start(d_e0[:, :], eT[:, :])
                    return (lb, st, nt, eT)

                def ctx_stage(state):
                    lb, st, nt, eT = state
                    last = (st == ST - 1)
                    for i in range(4):
                        for cc in range(4):
                            c0 = (cc * BP + lb) * H
                            nc.tensor.matmul(
                                ctx_ps[:, c0:c0 + H],
                                nt[:, i, cc * 128:(cc + 1) * 128],
                                eT[:, i * H:(i + 1) * H],
                                start=False, stop=(last and i == 3),
                                skip_group_check=True)
                        nc.tensor.matmul(
                            sums_ps[:H, lb:lb + 1],
                            eT[:, i * H:(i + 1) * H], ones_col[:, :1],
                            start=False, stop=(last and i == 3),
                            skip_group_check=True)
                    if last and VCHAIN:
                        value_chain(lb)

                def value_chain(lb):
                    # normalize lb's ctx columns and un-absorb values while
                    # later sequences' attention is still streaming
                    rec1 = smp.tile([H, 1], F32, tag="rec1", name=f"rec{lb}")
                    nc.vector.reciprocal(rec1[:, :], sums_ps[:H, lb:lb + 1])
                    dg = smp.tile([H, H], BF16, tag="diag16", name=f"dg{lb}")
                    nc.vector.tensor_scalar_mul(dg[:, :], ident_bf[:H, :H],
                                                rec1[:, :1])
                    bc1_ps = pssc.tile([128, H], F32, tag="sc",
                                       name=f"bc{lb}")
                    nc.tensor.matmul(bc1_ps[:, :], ones16[:H, :], dg[:, :],
                                     start=True, stop=True)
                    bc1 = smp.tile([128, H], BF16, tag="bcsb",
                                   name=f"bcs{lb}")
                    nc.scalar.copy(bc1[:, :], bc1_ps[:, :])
                    for cc in range(4):
                        nc.vector.tensor_tensor(
                            ctxn_sb[:, cc, lb, :],
                            ctx_ps[:, (cc * BP + lb) * H:
                                   (cc * BP + lb + 1) * H],
                            bc1[:, :], MULT)
                    pv = pssc.tile([128, H], F32, tag="sc",
                                   name=f"psv{lb}")
                    for h in range(H):
                        for cc in range(4):
                            nc.tensor.matmul(
                                pv[:, h:h + 1],
                                wvc_sb[:, h, cc, :],
                                ctxn_sb[:, cc, lb, h:h + 1],
                                start=(cc == 0), stop=(cc == 3))
                    nc.scalar.copy(ovT[:, :, lb], pv[:, :])
                    nc.scalar.dma_start(
                        ag_in[lb:lb + 1, :].rearrange("b (h p) -> p (b h)",
                                                      p=128),
                        ovT[:, :, lb])

                loaded, scored = [], []
                for k in range(len(steps) + D2 + 1):
                    if k < len(steps):
                        loaded.append(load_stage(k))
                    if k >= D1 and k - D1 < len(steps):
                        scored.append(score_stage(loaded[k - D1]))
                    if k >= D2 and k - D2 < len(steps):
                        ctx_stage(scored[k - D2 - (D1 - D1)][0]
                                  if False else scored[k - D2])

                if not VCHAIN:
                    for lb in range(BP):
                        value_chain(lb)
                if debug:
                    ds = smp.tile([H, BP], F32, tag="dsums")
                    nc.vector.tensor_copy(ds[:, :], sums_ps[:H, :])
                    nc.scalar.dma_start(d_sums[:, :], ds[:, :])
                    nc.scalar.dma_start(
                        d_ctxn[:, :],
                        ctxn_sb[:, :, :, :].rearrange("p c b h -> p (c b h)"))

            # ================= output =================
            with (
                tc.tile_pool(name="psot", bufs=1, space="PSUM") as psot,
                tc.tile_pool(name="psoo", bufs=1, space="PSUM") as psoo,
            ):
                # ag_in rows were written per-lb by the value chains
                if debug:
                    dov = qsb.tile([BP, H * DV], BF16)
                    nc.scalar.dma_start(dov[:, :], ag_in[:, :])
                    nc.scalar.dma_start(d_ov[:, :], dov[:, :])
                if fake_coll:
                    nc.gpsimd.dma_start(ag_out[0:BP, :], ag_in[:, :])
                else:
                    nc.gpsimd.collective_compute(
                        "AllGather", BYPASS, replica_groups=rg,
                        ins=[ag_in.opt()], outs=[ag_out.opt()])
                ov32_sb = qsb.tile([B, H * DV], BF16)
                nc.scalar.dma_start(ov32_sb[:, :], ag_out[:, :])
                of_ps = psot.tile([128, 16 * B], BF16, tag="ovT")
                for t in range(16):
                    nc.tensor.transpose(of_ps[:, t * B:(t + 1) * B],
                                        ov32_sb[:B, t * 128:(t + 1) * 128],
                                        ident_bf[:B, :B])
                ovT_f = qsb.tile([128, 16, B], BF16)
                nc.scalar.copy(ovT_f[:, :, :],
                               of_ps[:, :].rearrange("p (t b) -> p t b", t=16))
                po = psoo.tile([128, 5 * B], F32, tag="oproj")
                for c5 in range(5):
                    for t in range(16):
                        nc.tensor.matmul(
                            po[:, c5 * B:(c5 + 1) * B],
                            wo_sb[:, t, c5 * 128:(c5 + 1) * 128],
                            ovT_f[:, t, :],
                            start=(t == 0), stop=(t == 15))
                out_sb = qsb.tile([128, 5, B], BF16)
                nc.scalar.copy(out_sb[:, :, :],
                               po[:, :].rearrange("p (t b) -> p t b", t=5))
                nc.scalar.dma_start(
                    out[:, :].rearrange("(t p) b -> p t b", p=128),
                    out_sb[:, :, :])

    nc.compile()
    return nc


# ----------------------------- host wrapper ------------------------------


def _prep_in_maps(inputs, S, n_cores, tp=True):
    hidden = np.asarray(inputs["hidden_states"], np.float32)
    pos = np.asarray(inputs["positions"], np.int32)
    w_qkv_a = np.asarray(inputs["w_qkv_a"], np.float32)
    q_a_norm_w = np.asarray(inputs["q_a_norm_w"], np.float32)
    w_q_b = np.asarray(inputs["w_q_b"], np.float32)
    kv_a_norm_w = np.asarray(inputs["kv_a_norm_w"], np.float32)
    w_kc = np.asarray(inputs["w_kc"], np.float32)
    w_vc = np.asarray(inputs["w_vc"], np.float32)
    w_o = np.asarray(inputs["w_o"], np.float32)
    cache_l = np.asarray(inputs["kv_cache_latent"], np.float32)
    cache_r = np.asarray(inputs["kv_cache_rope"], np.float32)

    # current-token cache update (host, as in the original baseline)
    latent = hidden @ w_qkv_a[:, QL:QL + KL]
    k_pe = hidden @ w_qkv_a[:, QL + KL:]
    latent_n = _rmsnorm_np(latent, kv_a_norm_w)
    k_pe_r = _rope_np(k_pe.astype(np.float32), pos)
    cache_l = cache_l.copy()
    cache_r = cache_r.copy()
    cache_l[:, -1, :] = latent_n
    cache_r[:, -1, :] = k_pe_r
    cache_nat = cache_l[:, :S, :].astype(NPBF)
    cacheT_r = np.ascontiguousarray(
        cache_r[:, :S, :].transpose(0, 2, 1)).astype(NPBF)

    hiddenT = np.ascontiguousarray(hidden.T).astype(NPBF)
    RT = _rope_RT(pos).astype(NPBF)

    # folded q_b: rmsnorm weight + per-head w_kc absorption
    w_qb_n = q_a_norm_w[:, None] * w_q_b                  # [QL, H*(DN+DR)]
    w3 = w_qb_n.reshape(QL, H, DN + DR)
    wabs = np.einsum("qhd,hdc->qhc", w3[:, :, :DN], w_kc)  # [QL, H, KL]
    w_qbx = np.concatenate(
        [wabs.reshape(QL, H * KL), w3[:, :, DN:].reshape(QL, H * DR)],
        axis=1).astype(NPBF)                               # [QL, 9216]

    w_qa_q = np.ascontiguousarray(w_qkv_a[:, :QL]).astype(NPBF)

    # packed value / output weights
    wvc_p = np.ascontiguousarray(
        w_vc.reshape(H, 4, 128, DV).transpose(2, 0, 1, 3)
        .reshape(128, H * 4 * DV)).astype(NPBF)
    wo3 = w_o.reshape(16, 128, HID)

    in_maps = []
    for k in range(n_cores):
        b0 = k * BP
        m = {
            "hT": np.ascontiguousarray(hiddenT[k * KH:(k + 1) * KH, :]),
            "w_qa": np.ascontiguousarray(w_qa_q[k * KH:(k + 1) * KH, :]),
            "w_qbx": np.ascontiguousarray(w_qbx[k * QLS:(k + 1) * QLS, :]),
            "ropeRT": np.ascontiguousarray(RT[b0:b0 + BP]),
            "cache_nat": np.ascontiguousarray(cache_nat[b0:b0 + BP]),
            "cacheT_r": np.ascontiguousarray(cacheT_r[b0:b0 + BP]),
            "wvc_p": wvc_p,
            "wo_p": np.ascontiguousarray(
                wo3[:, :, k * HO:(k + 1) * HO].transpose(1, 0, 2)
                .reshape(128, 16 * HO)).astype(NPBF),
        }
        in_maps.append(m)
    return in_maps


def _unshard(results, tp=True):
    return np.concatenate(
        [np.asarray(results[k]["out"], np.float32).T
         for k in range(N_CORES)], axis=1)


def run(inputs, S=4096, trace=False):
    key = (S, N_CORES, TP)
    if key not in _CACHE:
        _CACHE[key] = _build(S, N_CORES, TP)
    nc = _CACHE[key]
    in_maps = _prep_in_maps(inputs, S, N_CORES, TP)
    res = bass_utils.run_bass_kernel_spmd(
        nc, in_maps, core_ids=list(range(N_CORES)), trace=trace)
    return _unshard(res.results, TP), res


def kernel(**inputs) -> np.ndarray:
    out, _ = run(inputs)
    return out.astype(np.float32)
